# revision 1
# baseline (speedup 1.0000x reference)
"""Trainium2 Bass kernel for nn_CustomizeLSTMCell.

reference:
    pre = w_in_input @ s_in + w_out_input @ s_out + u_in_input @ h_in + u_out_input @ h_out
    g = sigmoid(pre)
    cell_state = g * last_c + g * g          # = g * (last_c + g)
    hidden_state = g * cell_state
    returns (cell_state, hidden_state)       # each [H, B] f32

Sharding: pure data parallel along the batch (column) axis B across 8
NeuronCores; the four [128,128] weights are replicated (pre-transposed on
host so they can feed the PE as lhsT directly).
"""

import sys
from contextlib import ExitStack

import numpy as np

for _p in ("/opt/trn_rl_repo", "/opt/pypackages"):
    if _p not in sys.path:
        sys.path.append(_p)

import concourse.bass as bass
import concourse.tile as tile
from concourse import bacc, mybir
from concourse import bass_utils

H = 128
S = 128
B = 131072
N_CORES = 8
B_CORE = B // N_CORES  # 16384 columns per core

N_TILE = 2048  # columns per SBUF tile (1 MiB per [128, N_TILE] f32 DMA)
MM_FREE = 512  # matmul free dim = one PSUM bank of f32

F32 = mybir.dt.float32

BIG_INPUTS = ("s_in", "s_out", "h_in", "h_out", "last_c")
WEIGHTS = ("w_in_input", "w_out_input", "u_in_input", "u_out_input")
WEIGHT_T_NAMES = tuple(w + "_T" for w in WEIGHTS)


def emit_lstm_tile(ctx: ExitStack, tc: tile.TileContext, io: dict, b_core: int):
    nc = tc.nc
    n_tiles = b_core // N_TILE

    wpool = ctx.enter_context(tc.tile_pool(name="weights", bufs=1))
    inpool = ctx.enter_context(tc.tile_pool(name="inp", bufs=2))
    gpool = ctx.enter_context(tc.tile_pool(name="gate", bufs=2))
    opool = ctx.enter_context(tc.tile_pool(name="outs", bufs=2))
    psum = ctx.enter_context(tc.tile_pool(name="psum", bufs=8, space="PSUM"))

    wtiles = []
    for wname in WEIGHT_T_NAMES:
        wt = wpool.tile([S, H], F32, name=f"w_{wname}")
        nc.sync.dma_start(wt[:], io[wname][:])
        wtiles.append(wt)
    w_i, w_o, u_i, u_o = wtiles

    for i in range(n_tiles):
        ts = bass.ts(i, N_TILE)
        t_sin = inpool.tile([S, N_TILE], F32, name="t_sin")
        nc.sync.dma_start(t_sin[:], io["s_in"][:, ts])
        t_sout = inpool.tile([S, N_TILE], F32, name="t_sout")
        nc.sync.dma_start(t_sout[:], io["s_out"][:, ts])
        t_hin = inpool.tile([H, N_TILE], F32, name="t_hin")
        nc.sync.dma_start(t_hin[:], io["h_in"][:, ts])
        t_hout = inpool.tile([H, N_TILE], F32, name="t_hout")
        nc.sync.dma_start(t_hout[:], io["h_out"][:, ts])
        t_lc = inpool.tile([H, N_TILE], F32, name="t_lc")
        nc.sync.dma_start(t_lc[:], io["last_c"][:, ts])

        g = gpool.tile([H, N_TILE], F32, name="g")
        for j in range(N_TILE // MM_FREE):
            js = bass.ts(j, MM_FREE)
            ps = psum.tile([H, MM_FREE], F32, name="ps")
            nc.tensor.matmul(ps[:], w_i[:], t_sin[:, js], start=True, stop=False)
            nc.tensor.matmul(ps[:], w_o[:], t_sout[:, js], start=False, stop=False)
            nc.tensor.matmul(ps[:], u_i[:], t_hin[:, js], start=False, stop=False)
            nc.tensor.matmul(ps[:], u_o[:], t_hout[:, js], start=False, stop=True)
            nc.scalar.activation(
                g[:, js], ps[:], mybir.ActivationFunctionType.Sigmoid
            )

        # c = g * (last_c + g); h = g * c
        tmp = opool.tile([H, N_TILE], F32, name="tmp")
        nc.vector.tensor_add(tmp[:], g[:], t_lc[:])
        c = opool.tile([H, N_TILE], F32, name="c")
        nc.vector.tensor_mul(c[:], g[:], tmp[:])
        h = opool.tile([H, N_TILE], F32, name="h")
        nc.vector.tensor_mul(h[:], g[:], c[:])

        nc.sync.dma_start(io["cell_state"][:, ts], c[:])
        nc.sync.dma_start(io["hidden_state"][:, ts], h[:])


def build_model(b_core: int = B_CORE, n_cores: int = N_CORES):
    nc = bacc.Bacc(
        "TRN2",
        target_bir_lowering=False,
        debug=False,
        enable_asserts=False,
        num_devices=n_cores,
    )
    io = {}
    for name in BIG_INPUTS:
        io[name] = nc.dram_tensor(name, [S, b_core], F32, kind="ExternalInput").ap()
    for name in WEIGHT_T_NAMES:
        io[name] = nc.dram_tensor(name, [S, H], F32, kind="ExternalInput").ap()
    io["cell_state"] = nc.dram_tensor(
        "cell_state", [H, b_core], F32, kind="ExternalOutput"
    ).ap()
    io["hidden_state"] = nc.dram_tensor(
        "hidden_state", [H, b_core], F32, kind="ExternalOutput"
    ).ap()

    with tile.TileContext(nc) as tc, ExitStack() as ctx:
        emit_lstm_tile(ctx, tc, io, b_core)
    nc.compile()
    return nc


_model_cache: dict = {}


def _get_model():
    if "nc" not in _model_cache:
        _model_cache["nc"] = build_model()
    return _model_cache["nc"]


def make_in_maps(inputs: dict, b_core: int = B_CORE, n_cores: int = N_CORES):
    weights_t = {
        wname + "_T": np.ascontiguousarray(np.asarray(inputs[wname]).T)
        for wname in WEIGHTS
    }
    in_maps = []
    for c in range(n_cores):
        sl = slice(c * b_core, (c + 1) * b_core)
        m = {
            name: np.ascontiguousarray(np.asarray(inputs[name])[:, sl])
            for name in BIG_INPUTS
        }
        m.update(weights_t)
        in_maps.append(m)
    return in_maps


def run_spmd(inputs: dict, trace: bool = False, **kwargs):
    nc = _get_model()
    in_maps = make_in_maps(inputs)
    res = bass_utils.run_bass_kernel_spmd(
        nc, in_maps, core_ids=list(range(N_CORES)), trace=trace, **kwargs
    )
    cell = np.concatenate(
        [res.results[c]["cell_state"] for c in range(N_CORES)], axis=1
    )
    hidden = np.concatenate(
        [res.results[c]["hidden_state"] for c in range(N_CORES)], axis=1
    )
    return (cell, hidden), res


def kernel(**inputs):
    outs, _ = run_spmd(inputs, trace=False)
    return outs


# revision 4
# speedup vs baseline: 1.0949x; 1.0949x over previous
"""Trainium2 Bass kernel for nn_CustomizeLSTMCell.

reference:
    pre = w_in_input @ s_in + w_out_input @ s_out + u_in_input @ h_in + u_out_input @ h_out
    g = sigmoid(pre)
    cell_state = g * last_c + g * g          # = g * (last_c + g)
    hidden_state = g * cell_state
    returns (cell_state, hidden_state)       # each [H, B] f32

Sharding: pure data parallel along the batch (column) axis B across 8
NeuronCores; the four [128,128] weights are replicated (pre-transposed on
host so they can feed the PE as lhsT directly).
"""

import sys
from contextlib import ExitStack

import numpy as np

for _p in ("/opt/trn_rl_repo", "/opt/pypackages"):
    if _p not in sys.path:
        sys.path.append(_p)

import concourse.bass as bass
import concourse.tile as tile
from concourse import bacc, mybir
from concourse import bass_utils

H = 128
S = 128
B = 131072
N_CORES = 8
B_CORE = B // N_CORES  # 16384 columns per core

N_TILE = 2048  # columns per SBUF tile (1 MiB per [128, N_TILE] f32 DMA)
MM_FREE = 512  # matmul free dim = one PSUM bank of f32

F32 = mybir.dt.float32

BIG_INPUTS = ("s_in", "s_out", "h_in", "h_out", "last_c")
WEIGHTS = ("w_in_input", "w_out_input", "u_in_input", "u_out_input")
WEIGHT_T_NAMES = tuple(w + "_T" for w in WEIGHTS)


F32R = mybir.dt.float32r


def emit_lstm_tile(ctx: ExitStack, tc: tile.TileContext, io: dict, b_core: int):
    """Per-core body.

    - loads issue on the Sync HWDGE ring, stores on the Scalar HWDGE ring
      (separate rings avoid head-of-line blocking of loads behind stores
      whose data isn't computed yet)
    - matmuls run as float32r (full-rate fp32 streaming, N=512 >= 256)
    - per-512-column chunk pipeline: PE (4 accum matmuls) -> ACT sigmoid
      -> GpSimd add -> DVE mul -> DVE mul; store issue is delayed by one
      chunk so the Scalar engine never stalls waiting for DVE results.
    """
    nc = tc.nc
    n_tiles = b_core // N_TILE
    n_chunks = N_TILE // MM_FREE

    wpool = ctx.enter_context(tc.tile_pool(name="weights", bufs=1))
    inpool = ctx.enter_context(tc.tile_pool(name="inp", bufs=3))
    gpool = ctx.enter_context(tc.tile_pool(name="gate", bufs=4))
    tpool = ctx.enter_context(tc.tile_pool(name="tmps", bufs=4))
    cpool = ctx.enter_context(tc.tile_pool(name="couts", bufs=4))
    hpool = ctx.enter_context(tc.tile_pool(name="houts", bufs=4))
    psum = ctx.enter_context(tc.tile_pool(name="psum", bufs=8, space="PSUM"))

    wtiles = []
    for wname in WEIGHT_T_NAMES:
        wt = wpool.tile([S, H], F32R, name=f"w_{wname}")
        nc.sync.dma_start(wt[:], io[wname][:].bitcast(F32R))
        wtiles.append(wt)
    w_i, w_o, u_i, u_o = wtiles

    pending_stores = None  # (c_chunk, h_chunk, dram_col_slice)

    def flush_stores():
        nonlocal pending_stores
        if pending_stores is not None:
            pc, ph, sl = pending_stores
            nc.scalar.dma_start(io["cell_state"][:, sl], pc[:])
            nc.scalar.dma_start(io["hidden_state"][:, sl], ph[:])
            pending_stores = None

    for i in range(n_tiles):
        ts = bass.ts(i, N_TILE)
        t_sin = inpool.tile([S, N_TILE], F32R, name="t_sin")
        nc.sync.dma_start(t_sin[:], io["s_in"][:, ts].bitcast(F32R))
        t_sout = inpool.tile([S, N_TILE], F32R, name="t_sout")
        nc.sync.dma_start(t_sout[:], io["s_out"][:, ts].bitcast(F32R))
        t_hin = inpool.tile([H, N_TILE], F32R, name="t_hin")
        nc.sync.dma_start(t_hin[:], io["h_in"][:, ts].bitcast(F32R))
        t_hout = inpool.tile([H, N_TILE], F32R, name="t_hout")
        nc.sync.dma_start(t_hout[:], io["h_out"][:, ts].bitcast(F32R))
        t_lc = inpool.tile([H, N_TILE], F32, name="t_lc")
        nc.sync.dma_start(t_lc[:], io["last_c"][:, ts])

        for j in range(n_chunks):
            js = bass.ts(j, MM_FREE)
            ps = psum.tile([H, MM_FREE], F32, name="ps")
            nc.tensor.matmul(ps[:], w_i[:], t_sin[:, js], start=True, stop=False)
            nc.tensor.matmul(ps[:], w_o[:], t_sout[:, js], start=False, stop=False)
            nc.tensor.matmul(ps[:], u_i[:], t_hin[:, js], start=False, stop=False)
            nc.tensor.matmul(ps[:], u_o[:], t_hout[:, js], start=False, stop=True)
            g = gpool.tile([H, MM_FREE], F32, name="g")
            nc.scalar.activation(
                g[:], ps[:], mybir.ActivationFunctionType.Sigmoid
            )
            flush_stores()  # previous chunk's c/h are ready by now

            # c = g * (last_c + g); h = g * c
            tmp = tpool.tile([H, MM_FREE], F32, name="tmp")
            nc.gpsimd.tensor_add(tmp[:], g[:], t_lc[:, js])
            c = cpool.tile([H, MM_FREE], F32, name="c")
            nc.vector.tensor_mul(c[:], g[:], tmp[:])
            h = hpool.tile([H, MM_FREE], F32, name="h")
            nc.vector.tensor_mul(h[:], g[:], c[:])
            pending_stores = (c, h, bass.ts(i * n_chunks + j, MM_FREE))

    flush_stores()


def build_model(b_core: int = B_CORE, n_cores: int = N_CORES):
    nc = bacc.Bacc(
        "TRN2",
        target_bir_lowering=False,
        debug=False,
        enable_asserts=False,
        num_devices=n_cores,
    )
    io = {}
    for name in BIG_INPUTS:
        io[name] = nc.dram_tensor(name, [S, b_core], F32, kind="ExternalInput").ap()
    for name in WEIGHT_T_NAMES:
        io[name] = nc.dram_tensor(name, [S, H], F32, kind="ExternalInput").ap()
    io["cell_state"] = nc.dram_tensor(
        "cell_state", [H, b_core], F32, kind="ExternalOutput"
    ).ap()
    io["hidden_state"] = nc.dram_tensor(
        "hidden_state", [H, b_core], F32, kind="ExternalOutput"
    ).ap()

    with tile.TileContext(nc) as tc, ExitStack() as ctx:
        emit_lstm_tile(ctx, tc, io, b_core)
    nc.compile()
    return nc


_model_cache: dict = {}


def _get_model():
    if "nc" not in _model_cache:
        _model_cache["nc"] = build_model()
    return _model_cache["nc"]


def make_in_maps(inputs: dict, b_core: int = B_CORE, n_cores: int = N_CORES):
    weights_t = {
        wname + "_T": np.ascontiguousarray(np.asarray(inputs[wname]).T)
        for wname in WEIGHTS
    }
    in_maps = []
    for c in range(n_cores):
        sl = slice(c * b_core, (c + 1) * b_core)
        m = {
            name: np.ascontiguousarray(np.asarray(inputs[name])[:, sl])
            for name in BIG_INPUTS
        }
        m.update(weights_t)
        in_maps.append(m)
    return in_maps


def run_spmd(inputs: dict, trace: bool = False, **kwargs):
    nc = _get_model()
    in_maps = make_in_maps(inputs)
    res = bass_utils.run_bass_kernel_spmd(
        nc, in_maps, core_ids=list(range(N_CORES)), trace=trace, **kwargs
    )
    cell = np.concatenate(
        [res.results[c]["cell_state"] for c in range(N_CORES)], axis=1
    )
    hidden = np.concatenate(
        [res.results[c]["hidden_state"] for c in range(N_CORES)], axis=1
    )
    return (cell, hidden), res


def kernel(**inputs):
    outs, _ = run_spmd(inputs, trace=False)
    return outs


# revision 6
# speedup vs baseline: 1.1943x; 1.0908x over previous
"""Trainium2 Bass kernel for nn_CustomizeLSTMCell.

reference:
    pre = w_in_input @ s_in + w_out_input @ s_out + u_in_input @ h_in + u_out_input @ h_out
    g = sigmoid(pre)
    cell_state = g * last_c + g * g          # = g * (last_c + g)
    hidden_state = g * cell_state
    returns (cell_state, hidden_state)       # each [H, B] f32

Sharding: pure data parallel along the batch (column) axis B across 8
NeuronCores; the four [128,128] weights are replicated (pre-transposed on
host so they can feed the PE as lhsT directly).
"""

import sys
from contextlib import ExitStack

import numpy as np

for _p in ("/opt/trn_rl_repo", "/opt/pypackages"):
    if _p not in sys.path:
        sys.path.append(_p)

import concourse.bass as bass
import concourse.tile as tile
from concourse import bacc, mybir
from concourse import bass_utils

H = 128
S = 128
B = 131072
N_CORES = 8
B_CORE = B // N_CORES  # 16384 columns per core

N_TILE = 2048  # columns per SBUF tile (1 MiB per [128, N_TILE] f32 DMA)
MM_FREE = 512  # matmul free dim = one PSUM bank of f32

F32 = mybir.dt.float32

BIG_INPUTS = ("s_in", "s_out", "h_in", "h_out", "last_c")
WEIGHTS = ("w_in_input", "w_out_input", "u_in_input", "u_out_input")
WEIGHT_T_NAMES = tuple(w + "_T" for w in WEIGHTS)


F32R = mybir.dt.float32r


def emit_lstm_tile(ctx: ExitStack, tc: tile.TileContext, io: dict, b_core: int):
    """Per-core body.

    - loads issue on the Sync HWDGE ring, stores on the Scalar HWDGE ring
      (separate rings avoid head-of-line blocking of loads behind stores
      whose data isn't computed yet)
    - matmuls run as float32r (full-rate fp32 streaming, N=512 >= 256)
    - per-512-column chunk pipeline: PE (4 accum matmuls) -> ACT sigmoid
      -> GpSimd add -> DVE mul -> DVE mul; store issue is delayed by one
      chunk so the Scalar engine never stalls waiting for DVE results.
    """
    nc = tc.nc
    BLK = 2 * MM_FREE  # 1024-col elementwise/store block = 2 PSUM chunks
    n_tiles = b_core // N_TILE
    n_blocks = N_TILE // BLK

    wpool = ctx.enter_context(tc.tile_pool(name="weights", bufs=1))
    inpool = ctx.enter_context(tc.tile_pool(name="inp", bufs=3))
    gpool = ctx.enter_context(tc.tile_pool(name="gate", bufs=3))
    tpool = ctx.enter_context(tc.tile_pool(name="tmps", bufs=3))
    cpool = ctx.enter_context(tc.tile_pool(name="couts", bufs=3))
    hpool = ctx.enter_context(tc.tile_pool(name="houts", bufs=3))
    psum = ctx.enter_context(tc.tile_pool(name="psum", bufs=8, space="PSUM"))

    wtiles = []
    for wname in WEIGHT_T_NAMES:
        wt = wpool.tile([S, H], F32R, name=f"w_{wname}")
        nc.sync.dma_start(wt[:], io[wname][:].bitcast(F32R))
        wtiles.append(wt)
    w_i, w_o, u_i, u_o = wtiles

    pending_stores = None  # (c_chunk, h_chunk, dram_col_slice)

    def flush_stores():
        nonlocal pending_stores
        if pending_stores is not None:
            pc, ph, sl = pending_stores
            nc.scalar.dma_start(io["cell_state"][:, sl], pc[:])
            nc.scalar.dma_start(io["hidden_state"][:, sl], ph[:])
            pending_stores = None

    for i in range(n_tiles):
        ts = bass.ts(i, N_TILE)
        t_sin = inpool.tile([S, N_TILE], F32R, name="t_sin")
        nc.sync.dma_start(t_sin[:], io["s_in"][:, ts].bitcast(F32R))
        t_sout = inpool.tile([S, N_TILE], F32R, name="t_sout")
        nc.sync.dma_start(t_sout[:], io["s_out"][:, ts].bitcast(F32R))
        t_hin = inpool.tile([H, N_TILE], F32R, name="t_hin")
        nc.sync.dma_start(t_hin[:], io["h_in"][:, ts].bitcast(F32R))
        t_hout = inpool.tile([H, N_TILE], F32R, name="t_hout")
        nc.sync.dma_start(t_hout[:], io["h_out"][:, ts].bitcast(F32R))
        t_lc = inpool.tile([H, N_TILE], F32, name="t_lc")
        nc.sync.dma_start(t_lc[:], io["last_c"][:, ts])

        for b in range(n_blocks):
            g = gpool.tile([H, BLK], F32, name="g")
            for j in range(BLK // MM_FREE):
                js = bass.ts(b * 2 + j, MM_FREE)  # within the 2048 tile
                ps = psum.tile([H, MM_FREE], F32, name="ps")
                nc.tensor.matmul(ps[:], w_i[:], t_sin[:, js], start=True, stop=False)
                nc.tensor.matmul(ps[:], w_o[:], t_sout[:, js], start=False, stop=False)
                nc.tensor.matmul(ps[:], u_i[:], t_hin[:, js], start=False, stop=False)
                nc.tensor.matmul(ps[:], u_o[:], t_hout[:, js], start=False, stop=True)
                nc.scalar.activation(
                    g[:, bass.ts(j, MM_FREE)], ps[:],
                    mybir.ActivationFunctionType.Sigmoid,
                )
            flush_stores()  # previous block's c/h are ready by now

            # c = g * (last_c + g); h = g * c  -- all on DVE, back to back
            bs = bass.ts(b, BLK)
            tmp = tpool.tile([H, BLK], F32, name="tmp")
            nc.vector.tensor_add(tmp[:], g[:], t_lc[:, bs])
            c = cpool.tile([H, BLK], F32, name="c")
            nc.vector.tensor_mul(c[:], g[:], tmp[:])
            h = hpool.tile([H, BLK], F32, name="h")
            nc.vector.tensor_mul(h[:], g[:], c[:])
            pending_stores = (c, h, bass.ts(i * n_blocks + b, BLK))

    flush_stores()


def build_model(b_core: int = B_CORE, n_cores: int = N_CORES):
    nc = bacc.Bacc(
        "TRN2",
        target_bir_lowering=False,
        debug=False,
        enable_asserts=False,
        num_devices=n_cores,
    )
    io = {}
    for name in BIG_INPUTS:
        io[name] = nc.dram_tensor(name, [S, b_core], F32, kind="ExternalInput").ap()
    for name in WEIGHT_T_NAMES:
        io[name] = nc.dram_tensor(name, [S, H], F32, kind="ExternalInput").ap()
    io["cell_state"] = nc.dram_tensor(
        "cell_state", [H, b_core], F32, kind="ExternalOutput"
    ).ap()
    io["hidden_state"] = nc.dram_tensor(
        "hidden_state", [H, b_core], F32, kind="ExternalOutput"
    ).ap()

    with tile.TileContext(nc) as tc, ExitStack() as ctx:
        emit_lstm_tile(ctx, tc, io, b_core)
    nc.compile()
    return nc


_model_cache: dict = {}


def _get_model():
    if "nc" not in _model_cache:
        _model_cache["nc"] = build_model()
    return _model_cache["nc"]


def make_in_maps(inputs: dict, b_core: int = B_CORE, n_cores: int = N_CORES):
    weights_t = {
        wname + "_T": np.ascontiguousarray(np.asarray(inputs[wname]).T)
        for wname in WEIGHTS
    }
    in_maps = []
    for c in range(n_cores):
        sl = slice(c * b_core, (c + 1) * b_core)
        m = {
            name: np.ascontiguousarray(np.asarray(inputs[name])[:, sl])
            for name in BIG_INPUTS
        }
        m.update(weights_t)
        in_maps.append(m)
    return in_maps


def run_spmd(inputs: dict, trace: bool = False, **kwargs):
    nc = _get_model()
    in_maps = make_in_maps(inputs)
    res = bass_utils.run_bass_kernel_spmd(
        nc, in_maps, core_ids=list(range(N_CORES)), trace=trace, **kwargs
    )
    cell = np.concatenate(
        [res.results[c]["cell_state"] for c in range(N_CORES)], axis=1
    )
    hidden = np.concatenate(
        [res.results[c]["hidden_state"] for c in range(N_CORES)], axis=1
    )
    return (cell, hidden), res


def kernel(**inputs):
    outs, _ = run_spmd(inputs, trace=False)
    return outs
